# revision 1
# baseline (speedup 1.0000x reference)
"""Sparse top-k attention (talking-heads, rotary, mem-kv, l2-normed QK)
distributed over 8 Trainium2 NeuronCores.

Sharding: data-parallel over batch (2) x row-block-parallel over the
query/sequence dim (4 blocks of 256 rows) = 8 shards, one per core.
Each core computes K/V for the full sequence of its batch (cheap, avoids
any collective), and the full attention pipeline for its 256 query rows
across all 16 heads -- talking-heads mixes over heads, which stays local
under this sharding. Outputs are gathered/reassembled on host.
"""
import numpy as np
import jax
import jax.numpy as jnp
from functools import partial

B, N, DIM, H, DH = 2, 1024, 1024, 16, 64
ROT = 32
NUM_MEM = 2
TOPK = 64
QK_SCALE = 10.0
MASK_VAL = -np.finfo(np.float32).max
NCORES = 8
RB = N // 4  # row block per core (4 blocks per batch)


def _rotate_half(x):
    x = x.reshape(*x.shape[:-1], 2, x.shape[-1] // 2)
    x1, x2 = x[..., 0, :], x[..., 1, :]
    return jnp.concatenate((-x2, x1), axis=-1)


def _shard_fn(x_full, cosf, sinf, Wq, Wk, Wv, Wo, mem_k, mem_v, th_pre,
              th_post, row0):
    """Runs on one core. x_full: [N, DIM] (this batch). row0: scalar row
    offset of this core's 256-row query block. Returns (out_rows,
    pre_rows, post_rows)."""
    n = N

    def split_heads(t, rows):
        return t.reshape(rows, H, DH).transpose(1, 0, 2)  # [H, rows, DH]

    # K/V over the full sequence; Q only for our row block.
    x_rows = jax.lax.dynamic_slice(x_full, (row0, 0), (RB, DIM))
    q = split_heads(x_rows @ Wq, RB)          # [H, RB, DH]
    k = split_heads(x_full @ Wk, n)           # [H, N, DH]
    v = split_heads(x_full @ Wv, n)           # [H, N, DH]

    cos_q = jax.lax.dynamic_slice(cosf, (row0, 0), (RB, ROT))
    sin_q = jax.lax.dynamic_slice(sinf, (row0, 0), (RB, ROT))

    def rot(t, c, s):
        r = t[..., :ROT] * c + _rotate_half(t[..., :ROT]) * s
        return jnp.concatenate((r, t[..., ROT:]), axis=-1)

    q = rot(q, cos_q, sin_q)
    k = rot(k, cosf, sinf)
    v = rot(v, cosf, sinf)

    # prepend memory kv along j
    k = jnp.concatenate((mem_k, k), axis=1)   # [H, N+2, DH]
    v = jnp.concatenate((mem_v, v), axis=1)

    def l2norm(t):
        nn = jnp.sqrt(jnp.sum(t * t, axis=-1, keepdims=True))
        return t / jnp.maximum(nn, 1e-12)

    q, k = l2norm(q), l2norm(k)
    dots = jnp.einsum('hid,hjd->hij', q, k) * QK_SCALE  # [H, RB, N+2]
    pre = dots
    dots = jnp.einsum('hg,gij->hij', th_pre, dots)
    j = n + NUM_MEM
    i_idx = row0 + jnp.arange(RB)[:, None]
    j_idx = jnp.arange(j)[None, :]
    causal = (j_idx - NUM_MEM) > i_idx
    dots = jnp.where(causal[None], MASK_VAL, dots)
    top, _ = jax.lax.top_k(dots, TOPK)
    vk = top[..., -1:]
    dots = jnp.where(dots < vk, MASK_VAL, dots)
    attn = jax.nn.softmax(dots, axis=-1)
    post = attn
    attn = jnp.einsum('hg,gij->hij', th_post, attn)
    out = jnp.einsum('hij,hjd->hid', attn, v)   # [H, RB, DH]
    out = out.transpose(1, 0, 2).reshape(RB, H * DH)
    out = out @ Wo                               # [RB, DIM]
    return out, pre, post


_compiled = None


def _get_compiled():
    global _compiled
    if _compiled is None:
        _compiled = jax.pmap(_shard_fn, axis_name='c',
                             in_axes=(0,) * 12)
    return _compiled


def kernel(x, rotary_pos_emb, Wq, Wk, Wv, Wo, mem_k, mem_v, th_pre,
           th_post):
    x = np.asarray(x, np.float32)
    cosf = np.cos(np.asarray(rotary_pos_emb, np.float32))
    sinf = np.sin(np.asarray(rotary_pos_emb, np.float32))

    def rep(a):  # replicate a host array to all 8 cores
        a = np.asarray(a, np.float32)
        return np.broadcast_to(a[None], (NCORES,) + a.shape)

    # per-core: batch = c//4, row block = (c%4)*RB
    xs = np.stack([x[c // 4] for c in range(NCORES)])        # [8, N, DIM]
    row0 = np.array([(c % 4) * RB for c in range(NCORES)], np.int32)

    f = _get_compiled()
    out_s, pre_s, post_s = f(xs, rep(cosf), rep(sinf), rep(Wq), rep(Wk),
                             rep(Wv), rep(Wo), rep(mem_k), rep(mem_v),
                             rep(th_pre), rep(th_post), row0)
    out_s = np.asarray(out_s)    # [8, RB, DIM]
    pre_s = np.asarray(pre_s)    # [8, H, RB, N+2]
    post_s = np.asarray(post_s)

    out = np.zeros((B, N, DIM), np.float32)
    pre = np.zeros((B, H, N, N + NUM_MEM), np.float32)
    post = np.zeros((B, H, N, N + NUM_MEM), np.float32)
    for c in range(NCORES):
        b, r0 = c // 4, (c % 4) * RB
        out[b, r0:r0 + RB] = out_s[c]
        pre[b, :, r0:r0 + RB] = pre_s[c]
        post[b, :, r0:r0 + RB] = post_s[c]
    return out, pre, post
